# revision 43
# baseline (speedup 1.0000x reference)
"""Local2d (unshared-weight conv) Bass kernel for 8 trn2 NeuronCores.

Problem: input (64,64,32,32), weight (32,32,128,64,3,3), bias (128,32,32)
-> out (64,128,32,32).  K=3, stride 1, pad 1.

Sharding: spatial over h_out — core i handles output rows 4i..4i+3 and
reads the disjoint weight slice for those rows, plus a 6-row input halo
slab.

The kernel is DMA-bound (all traffic serializes through the shared DMA
engines at ~360B/ns), so the wire format is low precision against the
2e-2 tolerance (measured rel err of this exact scheme: 1.28e-2):
  - ALL weights and the input image travel as fp8 e3m4 (4 mantissa
    bits), scaled x32 / x2 on host to clear the e3m4 subnormal floor
    (min normal 0.25, max 15.5; zero saturation on this data);
  - the x64 product scale is divided back out in the PSUM merge;
  - bias and output travel as fp16 (PSUM accumulates fp32, host casts
    the output back to fp32).

DMA-byte trims:
  - matmuls that would touch an all-zero pad column (wo==0/kj==0,
    wo==31/kj==2) are skipped and their weight blocks never shipped;
  - pad columns of the input image are never transferred;
  - the weight stream is cut into groups with a tapered tail (8,...,8,
    4,2,1,1 locations), all weight-group buffers are resident (bufs=19)
    so the stream never stalls, and the output chunks ride the idle
    gpsimd/scalar queues so only a tiny compute chain trails the last
    weight byte.

Per output location (ho,wo), per valid kj, accumulating into one PSUM
group: one "paired" matmul, K=128 (partitions 0-63 = channels at ki=0,
64-127 = channels at ki=2; the image's upper half is the slab shifted
by two rows) and one "single" matmul, K=64 (channels at ki=1, read from
the image's lower half at row hol+1).  Merge per segment, split across
the idle engines: orow = ps8 * (1/64) on ACT, then += bias on DVE.
Stationary operand = per-location weights [K,128(o)], moving = input
columns [K,64(b)].  Host pre-transposes the weights so the contraction
dim lands on partitions with fully contiguous DMA.
"""

import numpy as np

B, C, O, KK, H, W = 64, 64, 128, 3, 32, 32
HO = WO = 32
NCORES = 8
RPC = HO // NCORES          # output rows per core
LOCS = RPC * WO             # locations per core

WSCALE = 32.0               # e3m4 weight scale (max |w|*32 ~ 7.2 < 15.5)
XSCALE = 2.0                # e3m4 input scale  (max |x|*2 ~ 10.2 < 15.5)

# location groups: one weight DMA (wq+ws) per group, tapered at the end
_GROUP_SIZES = [8] * 15 + [4, 2, 1, 1]


def _skip(wo, kj):
    return (wo == 0 and kj == 0) or (wo == WO - 1 and kj == 2)


def _group_plan():
    """[(loc_start, nlocs, [(loc, kj), ...]), ...] with block order shared
    by the host weight packer and the kernel builder."""
    plan, s = [], 0
    for n in _GROUP_SIZES:
        blks = []
        for loc in range(s, s + n):
            wo = loc % WO
            for kj in range(3):
                if not _skip(wo, kj):
                    blks.append((loc, kj))
        plan.append((s, n, blks))
        s += n
    assert s == LOCS
    return plan


_PLAN = _group_plan()
NBLK = sum(len(b) for _, _, b in _PLAN)

# last row: PSUM/merge segment starts -> width (rows 0-2 use 8 wide)
_SEG_START = {0: 8, 8: 8, 16: 8, 24: 4, 28: 2, 30: 2}
# output DMA chunks for the last row: wo -> chunk width
_OUT_CHUNK = {15: 16, 23: 8, 27: 4, 31: 4}


def _build_bass(mode="full", ngroups=None, mix=0, repeat=1):
    from concourse import bacc
    import concourse.mybir as mybir
    from concourse.tile import TileContext

    f16 = mybir.dt.float16
    f32 = mybir.dt.float32
    f8 = mybir.dt.float8e3
    nc = bacc.Bacc("TRN2", target_bir_lowering=False, debug=False,
                   num_devices=NCORES)

    # fp8 e3m4 input image (x2), dup baked on host, no pad columns:
    # full-width part: rows idx 0..3 (lower = slab rows 0..3, upper =
    # slab rows 2..5); x8b = lower-only slab row 4 (the upper half of
    # index 4 is never read, so its bytes are never shipped).
    x8_d = nc.dram_tensor("x8", (128, RPC, W, B), f8,
                          kind="ExternalInput").ap()
    x8b_d = nc.dram_tensor("x8b", (64, W, B), f8,
                           kind="ExternalInput").ap()
    # weights pre-arranged on host as one partition-major flat stream;
    # per-group slices are fully contiguous per partition.
    wq_d = nc.dram_tensor("wq", (128, NBLK * O), f8,
                          kind="ExternalInput").ap()
    ws_d = nc.dram_tensor("ws", (64, NBLK * O), f8,
                          kind="ExternalInput").ap()
    bias_d = nc.dram_tensor("bias", (O, LOCS), f16,
                            kind="ExternalInput").ap()
    out_d = nc.dram_tensor("out", (RPC, O, WO, B), f16,
                           kind="ExternalOutput").ap()

    with TileContext(nc) as tc:
        with tc.tile_pool(name="xslab", bufs=1) as xpool, \
             tc.tile_pool(name="wpool", bufs=19) as wpool, \
             tc.tile_pool(name="spool", bufs=19) as spool, \
             tc.tile_pool(name="opool", bufs=4) as opool, \
             tc.tile_pool(name="bpool", bufs=1) as bpool, \
             tc.tile_pool(name="psum", bufs=1, space="PSUM") as pspool:

            X8 = xpool.tile([128, RPC + 1, W + 2, B], f8)
            nc.sync.dma_start(X8[:, 0:RPC, 1:W + 1, :], x8_d)
            nc.sync.dma_start(X8[0:64, RPC, 1:W + 1, :], x8b_d)
            bias_t = bpool.tile([128, LOCS], f16)
            nc.scalar.dma_start(bias_t, bias_d)

            out_rows = {}
            groups = _PLAN if ngroups is None else _PLAN[:ngroups]
            for rep in range(repeat):
              off = 0
              for gi, (s, n, blks) in enumerate(groups):
                  nb = len(blks)
                  bidx = {lk: i for i, lk in enumerate(blks)}
                  wq = wpool.tile([128, nb, O], f8, tag="wq",
                                  name=f"wq{rep}_{gi}")
                  ws = spool.tile([64, nb, O], f8, tag="ws",
                                  name=f"ws{rep}_{gi}")
                  wq_src = wq_d[:, off * O:(off + nb) * O] \
                      .rearrange("p (n o) -> p n o", o=O)
                  ws_src = ws_d[:, off * O:(off + nb) * O] \
                      .rearrange("p (n o) -> p n o", o=O)
                  nc.sync.dma_start(wq, wq_src)
                  (nc.scalar if mix == 0 else nc.sync).dma_start(ws, ws_src)
                  off += nb

                  for loc in range(s, s + n):
                      hol, wo = divmod(loc, WO)
                      if wo == 0:
                          out_rows[hol] = opool.tile([128, WO, B], f16,
                                                     tag="orow",
                                                     name=f"orow{rep}_{hol}")
                      orow = out_rows[hol]

                      last_row = hol == RPC - 1
                      if last_row:
                          if wo in _SEG_START:
                              seg_w = _SEG_START[wo]
                              seg0 = wo
                      elif wo % 8 == 0:
                          seg_w, seg0 = 8, wo
                      if wo == seg0:
                          ps8 = pspool.tile([128, seg_w, B], f32,
                                            tag="ps8", bufs=8,
                                            name=f"ps8_{rep}_{loc}")
                      kjs = [kj for kj in range(3) if not _skip(wo, kj)]
                      h8 = ps8[:, wo - seg0, :]
                      # ki=1 single reads slab row hol+1 = X8 lower index
                      # hol+1 (the lower half carries rows 0..4)
                      xs = X8[0:64, hol + 1]
                      for i, kj in enumerate(kjs):
                          nc.tensor.matmul(h8, wq[:, bidx[(loc, kj)], :],
                                           X8[:, hol, wo + kj, :],
                                           start=(i == 0), stop=False)
                      for i, kj in enumerate(kjs):
                          nc.tensor.matmul(h8, ws[:, bidx[(loc, kj)], :],
                                           xs[:, wo + kj, :],
                                           start=False,
                                           stop=(i == len(kjs) - 1))
                      if wo - seg0 == seg_w - 1:
                          # orow = ps8/(WSCALE*XSCALE) + bias, split over
                          # the idle ACT engine (descale) and DVE (add) —
                          # walrus rejects the fused scalar_tensor_tensor
                          osl = orow[:, seg0:wo + 1, :]
                          mul_eng = nc.vector if (last_row and seg0 >= 24) \
                              else nc.scalar
                          if mul_eng is nc.scalar:
                              mul_eng.mul(osl, ps8, 1.0 / (WSCALE * XSCALE))
                          else:
                              mul_eng.tensor_scalar_mul(
                                  osl, ps8, 1.0 / (WSCALE * XSCALE))
                          nc.vector.tensor_tensor(
                              osl, osl,
                              bias_t[:, loc - (seg_w - 1):loc + 1, None]
                              .to_broadcast((128, seg_w, B)),
                              mybir.AluOpType.add)
                      # output DMAs ride the idle gpsimd (SWDGE) queue so
                      # their compute-dependent waits never block the weight
                      # streams on sync/scalar; the final chunks spread over
                      # scalar/sync (both idle by then).
                      if last_row:
                          cw = _OUT_CHUNK.get(wo)
                          if cw is not None:
                              eng = {27: nc.scalar,
                                     31: nc.sync}.get(wo, nc.gpsimd)
                              eng.dma_start(
                                  out_d[hol, :, wo - cw + 1:wo + 1, :],
                                  orow[:, wo - cw + 1:wo + 1, :])
                      elif wo == WO - 1:
                          nc.gpsimd.dma_start(out_d[hol], orow)
    nc.finalize()
    return nc


def _prep_inputs(input, weight, bias):
    import ml_dtypes
    f8x = ml_dtypes.float8_e3m4
    inp = np.ascontiguousarray(input, dtype=np.float32)
    wgt = np.ascontiguousarray(weight, dtype=np.float32)
    bis = np.ascontiguousarray(bias, dtype=np.float32)

    in2 = np.ascontiguousarray(inp.transpose(2, 3, 1, 0))        # [h,w,c,b]
    # paired fp8 blocks [ho,wo,kj,(ki0 c; ki2 c)=128,o], scaled x32
    wq_full = np.clip(
        wgt[:, :, :, :, (0, 2), :] * WSCALE, -15.5, 15.5) \
        .transpose(0, 1, 5, 4, 3, 2).reshape(HO, WO, 3, 128, O).astype(f8x)
    # fp8 single blocks [ho,wo,kj,c,o] (ki=1), same x32 scale
    ws_full = np.clip(wgt[:, :, :, :, 1, :] * WSCALE, -15.5, 15.5) \
        .transpose(0, 1, 4, 3, 2).astype(f8x)

    in_maps = []
    for core in range(NCORES):
        h0 = core * RPC
        # fp8 image: [128, 5, w, b]; lower = slab rows 0..4 x2 in e3m4,
        # upper = slab rows 2..5 (row index r holds slab row r+2)
        simg = np.zeros((64, RPC + 3, W, B), np.float32)
        for hp in range(RPC + 2):
            h = h0 - 1 + hp
            if 0 <= h < H:
                simg[:, hp] = in2[h].transpose(1, 0, 2)
        simg = simg.astype(np.float16).astype(np.float32) * XSCALE
        img = np.zeros((128, RPC, W, B), np.float32)
        img[0:64] = simg[:, 0:RPC]
        img[64:128] = simg[:, 2:RPC + 2]
        img = img.astype(f8x)
        imgb = simg[:, RPC].astype(f8x)
        wq_blocks = []
        ws_blocks = []
        for s, n, blks in _PLAN:
            for loc, kj in blks:
                hol, wo = divmod(loc, WO)
                wq_blocks.append(wq_full[h0 + hol, wo, kj])   # [128, O]
                ws_blocks.append(ws_full[h0 + hol, wo, kj])   # [64, O]
        wq_c = np.stack(wq_blocks)          # [NBLK, 128, O]
        ws_c = np.stack(ws_blocks)          # [NBLK, 64, O]
        in_maps.append({
            "x8": img,
            "x8b": imgb,
            "wq": np.ascontiguousarray(wq_c.transpose(1, 0, 2))
                .reshape(128, NBLK * O),
            "ws": np.ascontiguousarray(ws_c.transpose(1, 0, 2))
                .reshape(64, NBLK * O),
            "bias": np.ascontiguousarray(
                bis.reshape(O, HO, WO)[:, h0:h0 + RPC, :].reshape(O, LOCS))
                .astype(np.float16),
        })
    return in_maps


_RUN_KW = {}  # test.py can inject trace=True etc.
_LAST_RESULT = [None]
_NC_CACHE = [None]


def kernel(input, weight, bias):
    from concourse.bass_utils import run_bass_kernel_spmd

    in_maps = _prep_inputs(input, weight, bias)
    if _NC_CACHE[0] is None:
        _NC_CACHE[0] = _build_bass()
    nc = _NC_CACHE[0]
    res = run_bass_kernel_spmd(nc, in_maps, core_ids=list(range(NCORES)),
                               **_RUN_KW)
    _LAST_RESULT[0] = res
    arr = np.stack([r["out"] for r in res.results])   # [core,hol,o,wo,b]
    out = arr.astype(np.float32).transpose(4, 2, 0, 1, 3).reshape(B, O, HO, WO)
    return np.ascontiguousarray(out)


# revision 44
# speedup vs baseline: 1.0456x; 1.0456x over previous
"""Local2d (unshared-weight conv) Bass kernel for 8 trn2 NeuronCores.

Problem: input (64,64,32,32), weight (32,32,128,64,3,3), bias (128,32,32)
-> out (64,128,32,32).  K=3, stride 1, pad 1.

Sharding: spatial over h_out — core i handles output rows 4i..4i+3 and
reads the disjoint weight slice for those rows, plus a 6-row input halo
slab.

The kernel is DMA-bound (all traffic serializes through the shared DMA
engines at ~360B/ns), so the wire format is low precision against the
2e-2 tolerance (measured rel err of this exact scheme: 1.28e-2):
  - ALL weights and the input image travel as fp8 e3m4 (4 mantissa
    bits), scaled x32 / x2 on host to clear the e3m4 subnormal floor
    (min normal 0.25, max 15.5; zero saturation on this data);
  - the x64 product scale is divided back out in the PSUM merge;
  - bias and output travel as fp16 (PSUM accumulates fp32, host casts
    the output back to fp32).

DMA-byte trims:
  - matmuls that would touch an all-zero pad column (wo==0/kj==0,
    wo==31/kj==2) are skipped and their weight blocks never shipped;
  - pad columns of the input image are never transferred;
  - the weight stream is cut into groups with a tapered tail (8,...,8,
    4,2,1,1 locations), all weight-group buffers are resident (bufs=19)
    so the stream never stalls, and the output chunks ride the idle
    gpsimd/scalar queues so only a tiny compute chain trails the last
    weight byte.

Per output location (ho,wo), per valid kj, accumulating into one PSUM
group: one "paired" matmul, K=128 (partitions 0-63 = channels at ki=0,
64-127 = channels at ki=2; the image's upper half is the slab shifted
by two rows) and one "single" matmul, K=64 (channels at ki=1, read from
the image's lower half at row hol+1).  Merge per segment, split across
the idle engines: orow = ps8 * (1/64) on ACT, then += bias on DVE.
Stationary operand = per-location weights [K,128(o)], moving = input
columns [K,64(b)].  Host pre-transposes the weights so the contraction
dim lands on partitions with fully contiguous DMA.
"""

import numpy as np

B, C, O, KK, H, W = 64, 64, 128, 3, 32, 32
HO = WO = 32
NCORES = 8
RPC = HO // NCORES          # output rows per core
LOCS = RPC * WO             # locations per core

WSCALE = 32.0               # e3m4 weight scale (max |w|*32 ~ 7.2 < 15.5)
XSCALE = 2.0                # e3m4 input scale  (max |x|*2 ~ 10.2 < 15.5)

# location groups: one weight DMA (wq+ws) per group, tapered at the end
_GROUP_SIZES = [8] * 15 + [4, 2, 1, 1]


def _skip(wo, kj):
    return (wo == 0 and kj == 0) or (wo == WO - 1 and kj == 2)


def _group_plan():
    """[(loc_start, nlocs, [(loc, kj), ...]), ...] with block order shared
    by the host weight packer and the kernel builder."""
    plan, s = [], 0
    for n in _GROUP_SIZES:
        blks = []
        for loc in range(s, s + n):
            wo = loc % WO
            for kj in range(3):
                if not _skip(wo, kj):
                    blks.append((loc, kj))
        plan.append((s, n, blks))
        s += n
    assert s == LOCS
    return plan


_PLAN = _group_plan()
NBLK = sum(len(b) for _, _, b in _PLAN)

# last row: PSUM/merge segment starts -> width (rows 0-2 use 8 wide)
_SEG_START = {0: 8, 8: 8, 16: 8, 24: 4, 28: 2, 30: 2}
# output DMA chunks for the last row: wo -> chunk width
_OUT_CHUNK = {15: 16, 23: 8, 27: 4, 31: 4}


def _build_bass(mode="full", ngroups=None, mix=0, repeat=1):
    from concourse import bacc
    import concourse.mybir as mybir
    from concourse.tile import TileContext

    f16 = mybir.dt.float16
    f32 = mybir.dt.float32
    f8 = mybir.dt.float8e3
    nc = bacc.Bacc("TRN2", target_bir_lowering=False, debug=False,
                   num_devices=NCORES)

    # fp8 e3m4 input image (x2), no pad columns: only the 6 UNIQUE slab
    # rows travel (rows 0..4 -> lower half idx 0..4, row 5 -> upper half
    # idx 3); the upper half's shifted duplicates of rows 2..4 are
    # rebuilt on-chip by three cheap DVE copies (bitcast to u16 for the
    # packed fast path).
    x8_d = nc.dram_tensor("x8", (64, RPC + 2, W, B), f8,
                          kind="ExternalInput").ap()
    # weights pre-arranged on host as one partition-major flat stream;
    # per-group slices are fully contiguous per partition.
    wq_d = nc.dram_tensor("wq", (128, NBLK * O), f8,
                          kind="ExternalInput").ap()
    ws_d = nc.dram_tensor("ws", (64, NBLK * O), f8,
                          kind="ExternalInput").ap()
    bias_d = nc.dram_tensor("bias", (O, LOCS), f16,
                            kind="ExternalInput").ap()
    out_d = nc.dram_tensor("out", (RPC, O, WO, B), f16,
                           kind="ExternalOutput").ap()

    with TileContext(nc) as tc:
        with tc.tile_pool(name="xslab", bufs=1) as xpool, \
             tc.tile_pool(name="wpool", bufs=19) as wpool, \
             tc.tile_pool(name="spool", bufs=19) as spool, \
             tc.tile_pool(name="opool", bufs=4) as opool, \
             tc.tile_pool(name="bpool", bufs=1) as bpool, \
             tc.tile_pool(name="psum", bufs=1, space="PSUM") as pspool:

            X8 = xpool.tile([128, RPC + 1, W + 2, B], f8)
            u16 = mybir.dt.uint16
            nc.sync.dma_start(X8[0:64, :, 1:W + 1, :], x8_d[:, 0:RPC + 1])
            nc.sync.dma_start(X8[64:128, RPC - 1, 1:W + 1, :],
                              x8_d[:, RPC + 1])
            for j in range(RPC - 1):
                nc.vector.tensor_copy(
                    X8[64:128, j, 1:W + 1, :].bitcast(u16),
                    X8[0:64, j + 2, 1:W + 1, :].bitcast(u16))
            bias_t = bpool.tile([128, LOCS], f16)
            nc.scalar.dma_start(bias_t, bias_d)

            out_rows = {}
            groups = _PLAN if ngroups is None else _PLAN[:ngroups]
            for rep in range(repeat):
              off = 0
              for gi, (s, n, blks) in enumerate(groups):
                  nb = len(blks)
                  bidx = {lk: i for i, lk in enumerate(blks)}
                  wq = wpool.tile([128, nb, O], f8, tag="wq",
                                  name=f"wq{rep}_{gi}")
                  ws = spool.tile([64, nb, O], f8, tag="ws",
                                  name=f"ws{rep}_{gi}")
                  wq_src = wq_d[:, off * O:(off + nb) * O] \
                      .rearrange("p (n o) -> p n o", o=O)
                  ws_src = ws_d[:, off * O:(off + nb) * O] \
                      .rearrange("p (n o) -> p n o", o=O)
                  nc.sync.dma_start(wq, wq_src)
                  (nc.scalar if mix == 0 else nc.sync).dma_start(ws, ws_src)
                  off += nb

                  for loc in range(s, s + n):
                      hol, wo = divmod(loc, WO)
                      if wo == 0:
                          out_rows[hol] = opool.tile([128, WO, B], f16,
                                                     tag="orow",
                                                     name=f"orow{rep}_{hol}")
                      orow = out_rows[hol]

                      last_row = hol == RPC - 1
                      if last_row:
                          if wo in _SEG_START:
                              seg_w = _SEG_START[wo]
                              seg0 = wo
                      elif wo % 8 == 0:
                          seg_w, seg0 = 8, wo
                      if wo == seg0:
                          ps8 = pspool.tile([128, seg_w, B], f32,
                                            tag="ps8", bufs=8,
                                            name=f"ps8_{rep}_{loc}")
                      kjs = [kj for kj in range(3) if not _skip(wo, kj)]
                      h8 = ps8[:, wo - seg0, :]
                      # ki=1 single reads slab row hol+1 = X8 lower index
                      # hol+1 (the lower half carries rows 0..4)
                      xs = X8[0:64, hol + 1]
                      for i, kj in enumerate(kjs):
                          nc.tensor.matmul(h8, wq[:, bidx[(loc, kj)], :],
                                           X8[:, hol, wo + kj, :],
                                           start=(i == 0), stop=False)
                      for i, kj in enumerate(kjs):
                          nc.tensor.matmul(h8, ws[:, bidx[(loc, kj)], :],
                                           xs[:, wo + kj, :],
                                           start=False,
                                           stop=(i == len(kjs) - 1))
                      if wo - seg0 == seg_w - 1:
                          # orow = ps8/(WSCALE*XSCALE) + bias, split over
                          # the idle ACT engine (descale) and DVE (add) —
                          # walrus rejects the fused scalar_tensor_tensor
                          osl = orow[:, seg0:wo + 1, :]
                          mul_eng = nc.vector if (last_row and seg0 >= 24) \
                              else nc.scalar
                          if mul_eng is nc.scalar:
                              mul_eng.mul(osl, ps8, 1.0 / (WSCALE * XSCALE))
                          else:
                              mul_eng.tensor_scalar_mul(
                                  osl, ps8, 1.0 / (WSCALE * XSCALE))
                          nc.vector.tensor_tensor(
                              osl, osl,
                              bias_t[:, loc - (seg_w - 1):loc + 1, None]
                              .to_broadcast((128, seg_w, B)),
                              mybir.AluOpType.add)
                      # output DMAs ride the idle gpsimd (SWDGE) queue so
                      # their compute-dependent waits never block the weight
                      # streams on sync/scalar; the final chunks spread over
                      # scalar/sync (both idle by then).
                      if last_row:
                          cw = _OUT_CHUNK.get(wo)
                          if cw is not None:
                              eng = {27: nc.scalar,
                                     31: nc.sync}.get(wo, nc.gpsimd)
                              eng.dma_start(
                                  out_d[hol, :, wo - cw + 1:wo + 1, :],
                                  orow[:, wo - cw + 1:wo + 1, :])
                      elif wo == WO - 1:
                          nc.gpsimd.dma_start(out_d[hol], orow)
    nc.finalize()
    return nc


def _prep_inputs(input, weight, bias):
    import ml_dtypes
    f8x = ml_dtypes.float8_e3m4
    inp = np.ascontiguousarray(input, dtype=np.float32)
    wgt = np.ascontiguousarray(weight, dtype=np.float32)
    bis = np.ascontiguousarray(bias, dtype=np.float32)

    in2 = np.ascontiguousarray(inp.transpose(2, 3, 1, 0))        # [h,w,c,b]
    # paired fp8 blocks [ho,wo,kj,(ki0 c; ki2 c)=128,o], scaled x32
    wq_full = np.clip(
        wgt[:, :, :, :, (0, 2), :] * WSCALE, -15.5, 15.5) \
        .transpose(0, 1, 5, 4, 3, 2).reshape(HO, WO, 3, 128, O).astype(f8x)
    # fp8 single blocks [ho,wo,kj,c,o] (ki=1), same x32 scale
    ws_full = np.clip(wgt[:, :, :, :, 1, :] * WSCALE, -15.5, 15.5) \
        .transpose(0, 1, 4, 3, 2).astype(f8x)

    in_maps = []
    for core in range(NCORES):
        h0 = core * RPC
        # fp8 image: [128, 5, w, b]; lower = slab rows 0..4 x2 in e3m4,
        # upper = slab rows 2..5 (row index r holds slab row r+2)
        simg = np.zeros((64, RPC + 3, W, B), np.float32)
        for hp in range(RPC + 2):
            h = h0 - 1 + hp
            if 0 <= h < H:
                simg[:, hp] = in2[h].transpose(1, 0, 2)
        simg = simg.astype(np.float16).astype(np.float32) * XSCALE
        img = simg[:, 0:RPC + 2].astype(f8x)
        wq_blocks = []
        ws_blocks = []
        for s, n, blks in _PLAN:
            for loc, kj in blks:
                hol, wo = divmod(loc, WO)
                wq_blocks.append(wq_full[h0 + hol, wo, kj])   # [128, O]
                ws_blocks.append(ws_full[h0 + hol, wo, kj])   # [64, O]
        wq_c = np.stack(wq_blocks)          # [NBLK, 128, O]
        ws_c = np.stack(ws_blocks)          # [NBLK, 64, O]
        in_maps.append({
            "x8": img,
            "wq": np.ascontiguousarray(wq_c.transpose(1, 0, 2))
                .reshape(128, NBLK * O),
            "ws": np.ascontiguousarray(ws_c.transpose(1, 0, 2))
                .reshape(64, NBLK * O),
            "bias": np.ascontiguousarray(
                bis.reshape(O, HO, WO)[:, h0:h0 + RPC, :].reshape(O, LOCS))
                .astype(np.float16),
        })
    return in_maps


_RUN_KW = {}  # test.py can inject trace=True etc.
_LAST_RESULT = [None]
_NC_CACHE = [None]


def kernel(input, weight, bias):
    from concourse.bass_utils import run_bass_kernel_spmd

    in_maps = _prep_inputs(input, weight, bias)
    if _NC_CACHE[0] is None:
        _NC_CACHE[0] = _build_bass()
    nc = _NC_CACHE[0]
    res = run_bass_kernel_spmd(nc, in_maps, core_ids=list(range(NCORES)),
                               **_RUN_KW)
    _LAST_RESULT[0] = res
    arr = np.stack([r["out"] for r in res.results])   # [core,hol,o,wo,b]
    out = arr.astype(np.float32).transpose(4, 2, 0, 1, 3).reshape(B, O, HO, WO)
    return np.ascontiguousarray(out)
